# revision 2
# baseline (speedup 1.0000x reference)
"""BatchHardTripletLoss kernel for 8 Trainium2 NeuronCores.

Math (matches the jax reference):
  dist2[i,j] = |e1_i|^2 + |e2_j|^2 - 2 e1.e2 + 2*eps*(s1_i - s2_j) + D*eps^2
             = a[i] + v[i,j],   v[i,j] = b[j] - 2<e1_i, e2_j>
  pos_max[i] = sqrt(clip(a[i] + max_{j in pos} v[i,j], 0))
  neg_min[i] = sqrt(clip(a[i] + min_{j in neg} v[i,j], 0))
  loss = mean over POS anchors of relu(pos_max - neg_min + margin)

v2 architecture: PAIRWISE TOURNAMENT.  The drain of the [anchors x
cands] f32 distance matrix out of PSUM (DVE/Act at ~1 elem/cyc/
partition) was the wall in v1.  Here candidates are paired on the
host; for pair (u, v):   max(d_u, d_v) = d_v + relu(d_u - d_v)
and d_u - d_v = (b_u - b_v) - 2<e1, e2_u - e2_v> is itself ONE matmul
column.  Per PSUM group of 1024 pair-columns:
  phase A: K=2 bf16 bias-diff tails (start=True) + fp8 diff mains
  Act:     relu in-place on the PSUM group (PE never clears
           has_written, so the later accumulate still works)
  phase B: K=2 bf16 base-bias tails + fp8 base mains, all
           start=False -> accumulate d_v on top of relu(d_u - d_v)
  DVE:     one chained tensor_scalar max-accum per class segment
This HALVES the reduced stream (4096 pair-cols vs 8192 cols per
i-tile).  Neg class is sign-flipped so both classes are MAX chains.
4 PSUM groups (2 banks each) rotate; PE/Act/DVE pipeline across them.

Host: pos-first column sort, exact f64 row stats, pairing (self-pair
for odd class tails, -BIG dummy pad to 4096 pairs, odd columns peeled
to an exact host-side merge), fp8/bf16 packing, final sqrt/margin/mean
+ exact f64 remainder rows.
"""

import os
import sys

for _p in ("/opt/trn_rl_repo",):
    if _p not in sys.path:
        sys.path.insert(0, _p)

import numpy as np
import ml_dtypes

EPS = 1e-6
MARGIN = 0.2
B = 8192
D = 128
NCORES = 8
NPAIR = 4096          # pair-columns per core (all cores see all pairs)
GW = 1024             # pair-cols per PSUM group = 2 banks
NG = NPAIR // GW      # 4 groups per i-tile
BIG = 1.0e30

_programs = {}
LAST_RESULTS = None   # BassKernelResults of the most recent run (for profiling)


def _build_program(n_it: int, pairb: int):
    """Bass program for one core.

    n_it: i-tiles (of 128 anchors) per core.
    pairb: pos/neg boundary in pair-column space (pairs [0,pairb) are
      pos-class, [pairb, NPAIR) neg-class).
    """
    import concourse.bacc as bacc
    import concourse.tile as tile
    from concourse import mybir

    f32 = mybir.dt.float32
    bf16 = mybir.dt.bfloat16
    fp8 = mybir.dt.float8e4
    AOT = mybir.AluOpType
    AFT = mybir.ActivationFunctionType

    SH = n_it * 128

    nc = bacc.Bacc(None)
    e1t = nc.declare_dram_parameter("e1t", [D, SH], fp8, isOutput=False)
    rhsA = nc.declare_dram_parameter("rhsA", [D, NPAIR], fp8, isOutput=False)
    rhsB = nc.declare_dram_parameter("rhsB", [D, NPAIR], fp8, isOutput=False)
    # tails rows 0..3: biasA hi,lo,hi,lo ; rows 4..7: biasB hi,lo,hi,lo
    # cols [0:SH] = lhsT ones, [SH:] = bias values per pair-col.
    tails = nc.declare_dram_parameter("tails", [8, SH + NPAIR], bf16, isOutput=False)
    outp = nc.declare_dram_parameter("out", [128, 2 * n_it], f32, isOutput=True)

    def group_segs(g):
        """Class segments (lo, hi, is_pos) of group g in pair-col coords."""
        glo, ghi = g * GW, (g + 1) * GW
        segs = []
        if glo < pairb:
            segs.append((glo, min(ghi, pairb), True))
        if ghi > pairb:
            segs.append((max(glo, pairb), ghi, False))
        return segs

    with tile.TileContext(nc) as tc:
        with (
            tc.tile_pool(name="const", bufs=1) as cpool,
            tc.tile_pool(name="ps", bufs=4, space="PSUM") as pspool,
            tc.tile_pool(name="red", bufs=2) as redpool,
        ):
            # tails: strip s lands on partitions 32s..32s+1.  Split the
            # DMA across both HWDGE queues (2-partition dest = slow).
            tlsb = cpool.tile([128, SH + NPAIR], bf16, tag="tlsb")
            half = (SH + NPAIR) // 2
            for s in range(4):
                nc.sync.dma_start(
                    tlsb[32 * s:32 * s + 2, 0:half], tails[2 * s:2 * s + 2, 0:half]
                )
                nc.scalar.dma_start(
                    tlsb[32 * s:32 * s + 2, half:], tails[2 * s:2 * s + 2, half:]
                )
            e1sb = cpool.tile([D, SH], fp8, tag="e1sb")
            nc.sync.dma_start(e1sb[:], e1t[:])
            outsb = cpool.tile([128, 2 * n_it], f32, tag="outsb")
            trf = cpool.tile([128, GW], bf16, tag="trf")
            rhsAsb = cpool.tile([D, NPAIR], fp8, tag="rhsAsb")
            rhsBsb = cpool.tile([D, NPAIR], fp8, tag="rhsBsb")
            # chunked loads so group 0 can start early
            for g in range(NG):
                nc.scalar.dma_start(
                    rhsAsb[:, g * GW:(g + 1) * GW], rhsA[:, g * GW:(g + 1) * GW]
                )
                nc.sync.dma_start(
                    rhsBsb[:, g * GW:(g + 1) * GW], rhsB[:, g * GW:(g + 1) * GW]
                )

            for it in range(n_it):
                icols = slice(it * 128, (it + 1) * 128)
                w8 = e1sb[:, icols]
                chain = redpool.tile([128, 2], f32, tag="chain", name=f"chain_{it}")
                chain_used = {True: False, False: False}

                for g in range(NG):
                    gcol = g * GW
                    ps = pspool.tile([128, GW], f32, tag="ps", name=f"ps_{it}_{g}")
                    # phase A: bias-diff tails (start=True) + diff mains
                    for s in range(2):
                        j0 = SH + gcol + s * 512
                        nc.tensor.matmul(
                            ps[:, s * 512:(s + 1) * 512],
                            tlsb[32 * s:32 * s + 2, icols],
                            tlsb[32 * s:32 * s + 2, j0:j0 + 512],
                            start=True,
                            stop=False,
                            tile_position=(32 * s, 0),
                        )
                    for s in range(2):
                        nc.tensor.matmul(
                            ps[:, s * 512:(s + 1) * 512],
                            w8,
                            rhsAsb[:, gcol + s * 512:gcol + (s + 1) * 512],
                            start=False,
                            stop=True,
                        )
                    # relu in place (PSUM -> PSUM, has_written untouched)
                    nc.scalar.activation(ps[:], ps[:], AFT.Relu)
                    # phase B: base tails + base mains, accumulate onto relu
                    for s in range(2):
                        j0 = SH + gcol + s * 512
                        nc.tensor.matmul(
                            ps[:, s * 512:(s + 1) * 512],
                            tlsb[64 + 32 * s:64 + 32 * s + 2, icols],
                            tlsb[64 + 32 * s:64 + 32 * s + 2, j0:j0 + 512],
                            start=False,
                            stop=False,
                            tile_position=(64 + 32 * s, 0),
                            skip_group_check=True,
                        )
                    for s in range(2):
                        nc.tensor.matmul(
                            ps[:, s * 512:(s + 1) * 512],
                            w8,
                            rhsBsb[:, gcol + s * 512:gcol + (s + 1) * 512],
                            start=False,
                            stop=True,
                            skip_group_check=True,
                        )
                    # drain: chained max-accum per class segment
                    for lo, hi, is_pos in group_segs(g):
                        ll, lh = lo - gcol, hi - gcol
                        ci = 0 if is_pos else 1
                        nc.vector.tensor_scalar(
                            out=trf[:, ll:lh],
                            in0=ps[:, ll:lh],
                            scalar1=(chain[:, ci:ci + 1]
                                     if chain_used[is_pos] else -BIG),
                            scalar2=None,
                            op0=AOT.max,
                            op1=AOT.max,
                            accum_out=chain[:, ci:ci + 1],
                        )
                        chain_used[is_pos] = True
                nc.vector.tensor_copy(outsb[:, 2 * it:2 * it + 2], chain[:])
            nc.sync.dma_start(outp[:], outsb[:])
    nc.compile()
    return nc


def _host_prep(emb1, emb2, target):
    """Sort columns pos-first, build pairs, pack device operands.

    Returns (k, n_it, a, e1p, pairb, e1t8, rhsA8, rhsB8, tails, peeled)
    peeled: list of (col_vector_f64, bias_f64, is_pos) handled on host.
    """
    tpos = target == 1
    k = int(tpos.sum())
    perm = np.concatenate([np.nonzero(tpos)[0], np.nonzero(~tpos)[0]])
    e2s = emb2[perm].astype(np.float64)          # [B, D] sorted pos-first
    b = (e2s * e2s).sum(1) - (2.0 * EPS) * e2s.sum(1)

    nneg = B - k
    peel_pos = k % 2
    peel_neg = nneg % 2
    k2, n2 = k - peel_pos, nneg - peel_neg
    peeled = []
    if peel_pos:
        peeled.append((e2s[k - 1], b[k - 1], True))
    if peel_neg:
        peeled.append((e2s[B - 1], b[B - 1], False))

    npairs_pos = k2 // 2
    npairs_neg = n2 // 2
    ndum = NPAIR - npairs_pos - npairs_neg
    assert ndum >= 0

    # pair columns: pos pairs (u=2p, v=2p+1), then neg pairs, then dummies
    dA = np.zeros((NPAIR, D))                    # rhsA columns (diff side)
    dB = np.zeros((NPAIR, D))                    # rhsB columns (base side)
    bA = np.zeros(NPAIR)
    bB = np.full(NPAIR, -BIG)                    # dummies default -BIG
    # pos: A = d_u - d_v ; B = d_v
    u = e2s[0:k2:2]
    v = e2s[1:k2:2]
    dA[:npairs_pos] = u - v
    dB[:npairs_pos] = v
    bA[:npairs_pos] = b[0:k2:2] - b[1:k2:2]
    bB[:npairs_pos] = b[1:k2:2]
    # neg (sign-flipped): A = d_v - d_u ; B = -d_v
    nu = e2s[k:k + n2:2]
    nv = e2s[k + 1:k + n2:2]
    sl = slice(npairs_pos, npairs_pos + npairs_neg)
    dA[sl] = nv - nu
    dB[sl] = -nv
    bA[sl] = b[k + 1:k + n2:2] - b[k:k + n2:2]
    bB[sl] = -b[k + 1:k + n2:2]

    e1p = emb1[tpos]                             # [k, D] pos anchors
    e1d = e1p.astype(np.float64)
    a = (e1d * e1d).sum(1) + (2.0 * EPS) * e1d.sum(1) + D * EPS * EPS

    n_it = min(k // 1024, 8)
    ndev = n_it * 1024
    e1dev = e1p[:ndev]

    e1m2t = np.ascontiguousarray((-2.0 * e1dev).T)      # [D, ndev] f32
    e1t8 = e1m2t.astype(ml_dtypes.float8_e4m3)
    rhsA8 = np.ascontiguousarray(dA.T).astype(np.float32).astype(
        ml_dtypes.float8_e4m3)
    rhsB8 = np.ascontiguousarray(dB.T).astype(np.float32).astype(
        ml_dtypes.float8_e4m3)

    SH = n_it * 128
    tails = np.zeros((8, SH + NPAIR), dtype=ml_dtypes.bfloat16)
    tails[:, 0:SH] = 1.0
    for src, base in ((bA, 0), (bB, 4)):
        hi = src.astype(np.float32).astype(ml_dtypes.bfloat16)
        lo = (src.astype(np.float32) - hi.astype(np.float32)).astype(
            ml_dtypes.bfloat16)
        for s in range(2):
            tails[base + 2 * s + 0, SH:] = hi
            tails[base + 2 * s + 1, SH:] = lo
    pairb = npairs_pos
    return k, n_it, a, e1p, pairb, e1t8, rhsA8, rhsB8, tails, peeled


def _host_remainder(e1rem, emb2, target):
    """Exact f64 pos_max/neg_min contribution of the remainder anchors."""
    e1d = e1rem.astype(np.float64)
    e2d = emb2.astype(np.float64)
    sq = (
        (e1d * e1d).sum(1)[:, None]
        + (e2d * e2d).sum(1)[None, :]
        - 2.0 * (e1d @ e2d.T)
        + 2.0 * EPS * (e1d.sum(1)[:, None] - e2d.sum(1)[None, :])
        + D * EPS * EPS
    )
    dist = np.sqrt(np.clip(sq, 0.0, None))
    pos = target == 1
    pos_max = np.where(pos[None, :], dist, -np.inf).max(1)
    neg_min = np.where(~pos[None, :], dist, np.inf).min(1)
    return np.clip(pos_max - neg_min + MARGIN, 0.0, None).sum()


def _numpy_fallback(emb1, emb2, target):
    e1 = emb1.astype(np.float64)
    e2 = emb2.astype(np.float64)
    sq = (
        (e1 * e1).sum(1)[:, None]
        + (e2 * e2).sum(1)[None, :]
        - 2.0 * (e1 @ e2.T)
        + 2.0 * EPS * (e1.sum(1)[:, None] - e2.sum(1)[None, :])
        + D * EPS * EPS
    )
    dist = np.sqrt(np.clip(sq, 0.0, None))
    pos = target == 1
    neg = target == 0
    pos_max = np.where(pos[None, :], dist, -np.inf).max(1)
    neg_min = np.where(neg[None, :], dist, np.inf).min(1)
    per = np.maximum(pos_max - neg_min + MARGIN, 0.0)
    w = pos.astype(np.float64)
    return np.float32((per * w).sum() / w.sum())


def kernel(emb1, emb2, target):
    global LAST_RESULTS
    emb1 = np.asarray(emb1, dtype=np.float32)
    emb2 = np.asarray(emb2, dtype=np.float32)
    target = np.asarray(target)
    assert emb1.shape == (B, D) and emb2.shape == (B, D)

    k = int((target == 1).sum())
    if k < 1024 or k == B:
        return _numpy_fallback(emb1, emb2, target)

    (k, n_it, a, e1p, pairb, e1t8, rhsA8, rhsB8, tails,
     peeled) = _host_prep(emb1, emb2, target)
    ndev = n_it * 1024
    SH = n_it * 128

    nc = _programs.get((n_it, pairb))
    if nc is None:
        nc = _build_program(n_it, pairb)
        _programs[(n_it, pairb)] = nc

    from concourse.bass_utils import run_bass_kernel_spmd

    in_maps = [
        {
            "e1t": np.ascontiguousarray(e1t8[:, c * SH:(c + 1) * SH]),
            "rhsA": rhsA8,
            "rhsB": rhsB8,
            "tails": tails,
        }
        for c in range(NCORES)
    ]
    res = run_bass_kernel_spmd(nc, in_maps, core_ids=list(range(NCORES)))
    LAST_RESULTS = res

    Mp = np.concatenate(
        [np.asarray(res.results[c]["out"])[:, 0::2].T.reshape(-1)
         for c in range(NCORES)]
    ).astype(np.float64)
    Mn = np.concatenate(
        [np.asarray(res.results[c]["out"])[:, 1::2].T.reshape(-1)
         for c in range(NCORES)]
    ).astype(np.float64)

    # merge peeled columns exactly (host f64)
    e1d = e1p[:ndev].astype(np.float64)
    for col, bias, is_pos in peeled:
        vj = bias - 2.0 * (e1d @ col)
        if is_pos:
            Mp = np.maximum(Mp, vj)
        else:
            Mn = np.maximum(Mn, -vj)

    adev = a[:ndev]
    pos2 = np.clip(adev + Mp, 0.0, None)
    neg2 = np.clip(adev - Mn, 0.0, None)   # min v = -max(-v)
    per = np.clip(np.sqrt(pos2) - np.sqrt(neg2) + MARGIN, 0.0, None)
    total = per.sum()
    if ndev < k:
        total += _host_remainder(e1p[ndev:], emb2, target)
    return np.float32(total / k)
